# revision 1
# baseline (speedup 1.0000x reference)
"""Trainium2 Bass kernel for nn_Attention_2826088481156.

Dense transformer attention block:
    qkv = x @ W_qkv.T + b_qkv            [B,T,3,H,D]
    scores = q k^T * SCALE + log(clip(cutoffs, 1e-15))
    attn = softmax(scores)
    out  = (attn @ v) @ W_out.T + b_out

Sharding (8 NeuronCores): data-parallel over B (=2), tensor-parallel over
heads (16 heads -> 4 per core).  Each core computes the full attention for
its 4 heads and a partial output projection over its 256 channels; the
host sums the 4 partials per batch and adds the (host-folded) biases.

Key algebra used on device:
    softmax(s + log c) = (c * exp(s)) / sum_k(c * exp(s))   [no log, no max]
    attn @ [V | 1] gives both the weighted values and the softmax
    denominator (row r of the PSUM accumulator), so normalization is a
    reciprocal + partition-broadcast + multiply at the end.
    b_v and b_out never enter the nonlinearity; they are folded on host:
    y += W_out @ b_v + b_out.

Everything on device is computed in a transposed [channel, token] layout so
that every matmul has its contraction dim on partitions; the host feeds
pre-transposed fp16 inputs (layout prep is part of sharding).
"""

import numpy as np

import concourse.bass as bass
import concourse.tile as tile
from concourse import bacc, mybir
from concourse.bass_utils import run_bass_kernel_spmd
from concourse.bass_interp import get_hw_module

F16 = mybir.dt.float16
F32 = mybir.dt.float32
Exp = mybir.ActivationFunctionType.Exp

DIM = 1024
H = 16
D = 64
B = 2
T = 2048
SCALE = 0.125          # 1/sqrt(64)
HPC = 4                # heads per core
CH = HPC * D           # 256 channels per core
NCORES = 8

_cache = {}


def build_kernel(t=T, compile_hw=True, loop_reps=0, phase=5, opt=None):
    """Build (and bacc-compile) the single-core SPMD program.

    loop_reps > 0 wraps the whole body in a hardware loop (for timing:
    the body runs loop_reps times per NEFF execution)."""
    import os
    from contextlib import ExitStack, nullcontext
    _opt = dict(opt or {})
    for k in ("SW", "LAG", "ATTN_FIRST", "S_BUFS", "EVAC"):
        if f"K_{k}" in os.environ:
            _opt.setdefault(k, int(os.environ[f"K_{k}"]))
    nc = bacc.Bacc("TRN2", target_bir_lowering=False, debug=False,
                   num_devices=NCORES)

    n_cb = DIM // 128          # 8 contraction blocks for projections
    n_kb = t // 128            # key blocks
    QW = 1024 if t >= 1024 else t   # query chunk width
    n_qcc = t // QW            # query chunks

    xT = nc.dram_tensor("xT", [DIM, t], F16, kind="ExternalInput")
    cT = nc.dram_tensor("cT", [t, t], F16, kind="ExternalInput")
    wqkT = nc.dram_tensor("wqkT", [DIM, 2 * CH], F16, kind="ExternalInput")
    wvT = nc.dram_tensor("wvT", [DIM, CH], F16, kind="ExternalInput")
    woT = nc.dram_tensor("woT", [CH, DIM], F16, kind="ExternalInput")
    bqk = nc.dram_tensor("bqk", [128, 4], F32, kind="ExternalInput")
    yT = nc.dram_tensor("yT", [DIM, t], F32, kind="ExternalOutput")

    with tile.TileContext(nc) as tc:
        loop_ctx = tc.For_i(0, loop_reps, 1) if loop_reps else nullcontext()
        with loop_ctx, ExitStack() as ctx:
            const = ctx.enter_context(tc.tile_pool(name="const", bufs=1))
            qkp = ctx.enter_context(tc.tile_pool(name="qkT", bufs=1))
            vp = ctx.enter_context(tc.tile_pool(name="v65", bufs=1))
            otp = ctx.enter_context(tc.tile_pool(name="ot", bufs=1))

            wqk_sb = []
            wv_sb = []
            for cb in range(n_cb):
                w1 = const.tile([128, 2 * CH], F16, tag=f"wqk{cb}", name=f"wqk{cb}")
                nc.sync.dma_start(w1[:], wqkT[cb * 128:(cb + 1) * 128, :])
                wqk_sb.append(w1)
                w2 = const.tile([128, CH], F16, tag=f"wv{cb}", name=f"wv{cb}")
                nc.sync.dma_start(w2[:], wvT[cb * 128:(cb + 1) * 128, :])
                wv_sb.append(w2)
            wo_sb = []
            for j in range(2):
                w3 = const.tile([128, DIM], F16, tag=f"wo{j}", name=f"wo{j}")
                nc.sync.dma_start(w3[:], woT[j * 128:(j + 1) * 128, :])
                wo_sb.append(w3)
            bqk_sb = const.tile([128, 4], F32, tag="bqk")
            nc.sync.dma_start(bqk_sb[:], bqk[:, :])

            # qkT_sb[j]: j=0 Q heads 0-1, j=1 Q heads 2-3, j=2 K heads 0-1, j=3 K heads 2-3
            qkT_sb = [qkp.tile([128, t], F16, tag=f"qk{j}", name=f"qkT{j}") for j in range(4)]
            # v65_sb[tb][:, h, 0:64] = V head h rows tb; [:, h, 64] = 1.0
            v65_sb = [vp.tile([128, HPC, 65], F16, tag=f"v{tb}", name=f"v65_{tb}") for tb in range(n_kb)]
            # OT_sb[j]: normalized attention output^T, heads (2j, 2j+1)
            ot_sb = [otp.tile([128, t], F16, tag=f"ot{j}", name=f"ot{j}") for j in range(2)]

            with tc.tile_pool(name="xTp", bufs=1) as xp:
                xT_sb = []
                for cb in range(n_cb):
                    xt = xp.tile([128, t], F16, tag=f"x{cb}", name=f"xt{cb}")
                    nc.sync.dma_start(xt[:], xT[cb * 128:(cb + 1) * 128, :])
                    xT_sb.append(xt)

                # ---- Stage A: qk^T = W_qk @ x^T (+bias) ----
                with tc.tile_pool(name="psA", bufs=2, space="PSUM") as psA:
                    for ob in range(4):
                        for tbb in range(t // QW):
                            p = psA.tile([128, QW], F32, tag="pa", name="pa")
                            for cb in range(n_cb):
                                for ns in range(QW // 512):
                                    nc.tensor.matmul(
                                        p[:, ns * 512:(ns + 1) * 512],
                                        wqk_sb[cb][:, ob * 128:(ob + 1) * 128],
                                        xT_sb[cb][:, tbb * QW + ns * 512: tbb * QW + (ns + 1) * 512],
                                        start=(cb == 0), stop=(cb == n_cb - 1))
                            nc.vector.tensor_scalar_add(
                                qkT_sb[ob][:, tbb * QW:(tbb + 1) * QW],
                                p[:], bqk_sb[:, ob:ob + 1])

                # ---- Stage B: V = x @ W_v^T (natural layout, interleaved 65) ----
                with tc.tile_pool(name="psB", bufs=2, space="PSUM") as psB:
                    for tb in range(n_kb):
                        p = psB.tile([128, CH], F32, tag="pb", name="pb")
                        for cb in range(n_cb):
                            nc.tensor.matmul(
                                p[:], xT_sb[cb][:, tb * 128:(tb + 1) * 128],
                                wv_sb[cb][:], start=(cb == 0), stop=(cb == n_cb - 1))
                        nc.vector.memset(v65_sb[tb][:, :, 64:65], 1.0)
                        nc.vector.tensor_copy(
                            v65_sb[tb][:, :, 0:64],
                            p[:].rearrange("p (h d) -> p h d", d=D))

            # ---- Stage C: attention + output projection ----
            # tunables (see perf notes): S-tile width, pipeline lag, emission
            # order of the trailing attn@V relative to scores
            SW = int(_opt.get("SW", 1024))       # scores tile width (psum)
            LAG = int(_opt.get("LAG", 2))        # attn@V trails by LAG S-tiles
            ATTN_FIRST = int(_opt.get("ATTN_FIRST", 0))
            S_BUFS = int(_opt.get("S_BUFS", 2))
            EVAC = int(_opt.get("EVAC", 0))
            # perf-probe: 0=real, 1=no psum accumulation, 2=const rhs (no
            # DVE->PE dep), 3=both  (1-3 give wrong results; timing only)
            ATTN_MODE = int(_opt.get("ATTN_MODE", 0))
            n_st = QW // SW                      # S-tiles per q-chunk per kb
            with tc.tile_pool(name="cTp", bufs=min(n_kb + 4, 2 * n_kb)) as cp, \
                 tc.tile_pool(name="ep", bufs=LAG + 2) as ep, \
                 tc.tile_pool(name="pp", bufs=LAG + 3) as pp, \
                 tc.tile_pool(name="orp", bufs=3) as orp, \
                 tc.tile_pool(name="rp", bufs=2) as rp, \
                 tc.tile_pool(name="rbp", bufs=2) as rbp, \
                 tc.tile_pool(name="tmpp", bufs=2) as tmpp, \
                 tc.tile_pool(name="yp", bufs=4) as yp, \
                 tc.tile_pool(name="psS", bufs=S_BUFS, space="PSUM") as psS, \
                 tc.tile_pool(name="psO", bufs=2, space="PSUM") as psO:

                for qcc in range(n_qcc):
                    q0 = qcc * QW
                    cT_tiles = []
                    for kb in range(n_kb):
                        ct = cp.tile([128, QW], F16, tag="ct", name=f"ct{kb}")
                        nc.sync.dma_start(ct[:], cT[kb * 128:(kb + 1) * 128, q0:q0 + QW])
                        cT_tiles.append(ct)

                    for h in range(HPC if phase >= 1 else 0):
                        j, bp = h // 2, (h % 2) * 64
                        O = psO.tile([128, QW], F32, tag="O", name="O")
                        # software pipeline over S-tiles (kb, ns): attn@V for
                        # tile i-LAG is issued before scores tile i so the
                        # in-order PE always has ready work
                        p_tiles = {}
                        n_tiles = n_kb * n_st
                        for i in range(n_tiles + LAG):
                            work = []
                            if i >= LAG:
                                work.append(("attn", i - LAG))
                            if i < n_tiles:
                                work.append(("scores", i))
                            if not ATTN_FIRST:
                                work.reverse()
                            for kind, ii in work:
                                kb, ns = divmod(ii, n_st)
                                if kind == "scores":
                                    S = psS.tile([128, SW], F32, tag="S", name="S")
                                    for ms in range(SW // 512):
                                        nc.tensor.matmul(
                                            S[:, ms * 512:(ms + 1) * 512],
                                            qkT_sb[2 + j][bp:bp + 64, kb * 128:(kb + 1) * 128],
                                            qkT_sb[j][bp:bp + 64,
                                                      q0 + ns * SW + ms * 512:
                                                      q0 + ns * SW + (ms + 1) * 512],
                                            start=True, stop=True)
                                    if phase >= 2:
                                        E = ep.tile([128, SW], F16, tag="E", name="E")
                                        nc.scalar.activation(E[:], S[:], Exp, scale=SCALE)
                                    if phase >= 3:
                                        P = pp.tile([128, SW], F16, tag="P", name="P")
                                        nc.vector.tensor_mul(
                                            P[:], E[:],
                                            cT_tiles[kb][:, ns * SW:(ns + 1) * SW])
                                        p_tiles[ii] = P
                                elif phase >= 4:
                                    P = p_tiles.pop(ii)
                                    no_acc = ATTN_MODE in (1, 3)
                                    if ATTN_MODE == 4:     # perf probe: groups of 4
                                        st_, sp_ = (kb % 4 == 0), (kb % 4 == 3 or kb == n_kb - 1)
                                    elif no_acc:
                                        st_, sp_ = True, True
                                    else:
                                        st_, sp_ = (kb == 0), (kb == n_kb - 1)
                                    for ms in range(SW // 512):
                                        nc.tensor.matmul(
                                            O[0:65, ns * SW + ms * 512:
                                              ns * SW + (ms + 1) * 512],
                                            v65_sb[kb][:, h, :],
                                            cT_tiles[kb][:, (ns * SW + ms * 512) % QW:
                                                         (ns * SW + ms * 512) % QW + 512]
                                            if ATTN_MODE in (2, 3)
                                            else P[:, ms * 512:(ms + 1) * 512],
                                            start=st_, stop=sp_)
                        if phase < 4:
                            continue
                        if EVAC:
                            # evacuate the PSUM bank with one fast copy; the
                            # normalization chain then runs from SBUF without
                            # blocking the next head's attn@V matmuls
                            oraw = orp.tile([65, QW], F32, tag="oraw", name="oraw")
                            nc.vector.tensor_copy(oraw[:], O[0:65, :])
                            osrc = oraw
                        else:
                            osrc = O
                        rr = rp.tile([1, QW], F32, tag="rr", name="rr")
                        nc.vector.reciprocal(rr[:], osrc[64:65, :])
                        rb = rbp.tile([64, QW], F32, tag="rb", name="rb")
                        nc.gpsimd.partition_broadcast(rb[:], rr[:])
                        if bp == 0:
                            nc.vector.tensor_mul(ot_sb[j][0:64, q0:q0 + QW],
                                                 osrc[0:64, :], rb[:])
                        else:
                            tmp = tmpp.tile([64, QW], F16, tag="tmp", name="tmp")
                            nc.vector.tensor_mul(tmp[:], osrc[0:64, :], rb[:])
                            nc.sync.dma_start(ot_sb[j][64:128, q0:q0 + QW], tmp[:])

                    # output projection for this query chunk
                    for ob in range(8 if phase >= 5 else 0):
                        Y = psO.tile([128, QW], F32, tag="O", name="Y")
                        for cb in range(2):
                            for ns in range(QW // 512):
                                nc.tensor.matmul(
                                    Y[:, ns * 512:(ns + 1) * 512],
                                    wo_sb[cb][:, ob * 128:(ob + 1) * 128],
                                    ot_sb[cb][:, q0 + ns * 512:q0 + (ns + 1) * 512],
                                    start=(cb == 0), stop=(cb == 1))
                        ys = yp.tile([128, QW], F32, tag="y", name="ys")
                        nc.vector.tensor_copy(ys[:], Y[:])
                        nc.sync.dma_start(yT[ob * 128:(ob + 1) * 128, q0:q0 + QW], ys[:])

    nc.compile()
    if compile_hw:
        nc.m = get_hw_module(nc.m)
    return nc


def make_in_maps(x, cutoffs, W_qkv, b_qkv, W_out):
    """Host-side sharding: slice + transpose + fp16 cast per core."""
    per_batch = []
    for b in range(B):
        xT_b = np.ascontiguousarray(x[b].T).astype(np.float16)
        cT_b = np.ascontiguousarray(cutoffs[b].T).astype(np.float16)
        per_batch.append((xT_b, cT_b))
    in_maps = []
    for core in range(NCORES):
        b, hg = core // HPC, core % HPC
        ch = slice(hg * CH, (hg + 1) * CH)
        chk = slice(DIM + hg * CH, DIM + (hg + 1) * CH)
        chv = slice(2 * DIM + hg * CH, 2 * DIM + (hg + 1) * CH)
        wqkT = np.ascontiguousarray(
            np.concatenate([W_qkv[ch], W_qkv[chk]], axis=0).T).astype(np.float16)
        wvT = np.ascontiguousarray(W_qkv[chv].T).astype(np.float16)
        woT = np.ascontiguousarray(W_out[:, ch].T).astype(np.float16)
        bqk_pp = np.concatenate([b_qkv[ch], b_qkv[chk]]).reshape(4, 128).T
        in_maps.append({
            "xT": per_batch[b][0], "cT": per_batch[b][1],
            "wqkT": wqkT, "wvT": wvT, "woT": woT,
            "bqk": np.ascontiguousarray(bqk_pp).astype(np.float32),
        })
    return in_maps


def kernel(x, cutoffs, W_qkv, b_qkv, W_out, b_out):
    x = np.asarray(x, dtype=np.float32)
    cutoffs = np.asarray(cutoffs, dtype=np.float32)
    W_qkv = np.asarray(W_qkv, dtype=np.float32)
    b_qkv = np.asarray(b_qkv, dtype=np.float32)
    W_out = np.asarray(W_out, dtype=np.float32)
    b_out = np.asarray(b_out, dtype=np.float32)

    if "nc" not in _cache:
        _cache["nc"] = build_kernel()
    nc = _cache["nc"]

    in_maps = make_in_maps(x, cutoffs, W_qkv, b_qkv, W_out)
    res = None
    last_err = None
    for attempt in range(3):
        try:
            res = run_bass_kernel_spmd(nc, in_maps, core_ids=list(range(NCORES)),
                                       trace=False)
            break
        except Exception as e:  # transient NRT/axon failures: retry
            last_err = e
            import time
            time.sleep(5)
    if res is None:
        raise last_err

    y = np.zeros((B, T, DIM), dtype=np.float32)
    for core in range(NCORES):
        b = core // HPC
        y[b] += res.results[core]["yT"].T
    bias_vec = W_out @ b_qkv[2 * DIM:] + b_out
    y += bias_vec[None, None, :]
    return y



# revision 15
# speedup vs baseline: 1.2369x; 1.2369x over previous
"""Trainium2 Bass kernel for nn_Attention_2826088481156 (v2).

Dense transformer attention block:
    qkv = x @ W_qkv.T + b_qkv            [B,T,3,H,D]
    scores = q k^T * SCALE + log(clip(cutoffs, 1e-15))
    attn = softmax(scores)
    out  = (attn @ v) @ W_out.T + b_out

Sharding (8 NeuronCores): data-parallel over B (=2), tensor-parallel over
heads (16 heads -> 4 per core).  Each core computes the full attention for
its 4 heads and a partial output projection over its 256 channels; the
host sums the 4 partials per batch and adds the (host-folded) biases.

Key algebra on device:
    softmax(s + log c) = (c * exp(s)) / sum_k(c * exp(s))   [no log, no max]
    attn @ [V | 1] gives the weighted values and the softmax denominator
    (row 64 of the PSUM accumulator) in one pass.
    b_v and b_out never enter the nonlinearity; host folds y += W_out@b_v + b_out.
    sqrt(1/8) scaling folded into W_q/W_k/b_q/b_k host-side.

v2 structural changes vs the 388us v1 baseline (engine-level findings from
real NTFF traces):
  * ACT (exp over 16.8M scores/core) is the 141us floor; everything else
    must hide underneath it, and the PE must stay busy to hold its 2.4GHz
    p-state (idle gaps drop it to 1.2GHz - measured 427ns vs 216ns per
    512-col matmul).
  * P = E * cutoffs alternates between DVE and the otherwise-idle
    Pool/GPSIMD engine (DVE was 194us busy in v1; fp8 everywhere was
    tried and rejected: absmax error 2-6% vs the 2% budget).
  * v1 spent 52us in 8 single-partition DVE RECIPROCALs; the softmax
    denominators are now DMA'd into one [4, QW] staging tile per query
    chunk -> ONE batched reciprocal (engines can only address partition
    bases 0/32/64/96, so rows are re-staged to partition 0 by tiny DMAs
    before the gpsimd broadcast).
  * attention-output PSUM is evacuated by a fast copy (psO runs with a
    single buffer) and the out-projection gets a dedicated PSUM bank.
  * stage B (V projection), most of stage A, and the qcc0 out-projection
    are emitted as PE filler work inside the attention pipeline so the
    in-order PE stream always has independent work while ACT chews
    (keeps the p-state up AND hides ~60us of projection work).
"""

import numpy as np

import concourse.bass as bass
import concourse.tile as tile
from concourse import bacc, mybir
from concourse.bass_utils import run_bass_kernel_spmd
from concourse.bass_interp import get_hw_module

F16 = mybir.dt.float16
F32 = mybir.dt.float32
Exp = mybir.ActivationFunctionType.Exp

DIM = 1024
H = 16
D = 64
B = 2
T = 2048
RSCALE = 0.125 ** 0.5   # folded into W_q, W_k, b_q, b_k on host
HPC = 4                 # heads per core
CH = HPC * D            # 256 channels per core
NCORES = 8

_cache = {}


def build_kernel(t=T, compile_hw=True, loop_reps=0, phase=5, opt=None):
    import os
    from collections import deque
    from contextlib import ExitStack, nullcontext
    _opt = dict(opt or {})
    for k in ("LAG", "FILL", "SERIAL_AB", "PMODE"):
        if f"K_{k}" in os.environ:
            _opt.setdefault(k, int(os.environ[f"K_{k}"]))
    LAG = int(_opt.get("LAG", 3))        # attnV trails scores by LAG kb tiles
    FILL = int(_opt.get("FILL", 2))      # filler chunks per pipeline step
    SERIAL_AB = int(_opt.get("SERIAL_AB", 0))
    PMODE = int(_opt.get("PMODE", 3))    # P-mult: 0=DVE, 1=Pool, 2=50/50, 3=25% Pool

    nc = bacc.Bacc("TRN2", target_bir_lowering=False, debug=False,
                   num_devices=NCORES)

    n_cb = DIM // 128           # 8 contraction blocks for projections
    n_kb = t // 128             # 16 key blocks
    QW = 1024 if t >= 1024 else t
    n_qcc = t // QW
    n_t2 = t // 512             # 512-token chunks for stage A

    xT = nc.dram_tensor("xT", [DIM, t], F16, kind="ExternalInput")
    cT = nc.dram_tensor("cT", [t, t], F16, kind="ExternalInput")
    wqkT = nc.dram_tensor("wqkT", [DIM, 512], F16, kind="ExternalInput")
    wvT = nc.dram_tensor("wvT", [DIM, CH], F16, kind="ExternalInput")
    woT = nc.dram_tensor("woT", [CH, DIM], F16, kind="ExternalInput")
    bqk = nc.dram_tensor("bqk", [128, 4], F32, kind="ExternalInput")
    yT = nc.dram_tensor("yT", [DIM, t], F16, kind="ExternalOutput")

    with tile.TileContext(nc) as tc:
        loop_ctx = tc.For_i(0, loop_reps, 1) if loop_reps else nullcontext()
        with loop_ctx, ExitStack() as ctx:
            const = ctx.enter_context(tc.tile_pool(name="const", bufs=1))
            qkp = ctx.enter_context(tc.tile_pool(name="qkT", bufs=1))
            vp = ctx.enter_context(tc.tile_pool(name="v65", bufs=1))
            otp = ctx.enter_context(tc.tile_pool(name="ot", bufs=1))
            dsp = ctx.enter_context(tc.tile_pool(name="ds", bufs=2))
            xp = ctx.enter_context(tc.tile_pool(name="xTp", bufs=1))

            wqk_sb = []
            wv_sb = []
            for cb in range(n_cb):
                w1 = const.tile([128, 512], F16, tag=f"wqk{cb}", name=f"wqk{cb}")
                nc.sync.dma_start(w1[:], wqkT[cb * 128:(cb + 1) * 128, :])
                wqk_sb.append(w1)
                w2 = const.tile([128, CH], F16, tag=f"wv{cb}", name=f"wv{cb}")
                nc.sync.dma_start(w2[:], wvT[cb * 128:(cb + 1) * 128, :])
                wv_sb.append(w2)
            wo_sb = []
            for j in range(2):
                w3 = const.tile([128, DIM], F16, tag=f"wo{j}", name=f"wo{j}")
                nc.sync.dma_start(w3[:], woT[j * 128:(j + 1) * 128, :])
                wo_sb.append(w3)
            bqk_sb = const.tile([128, 4], F32, tag="bqk")
            nc.sync.dma_start(bqk_sb[:], bqk[:, :])

            # qkT_sb[j]: j=0 Q heads 0-1, j=1 Q heads 2-3, j=2/3 same for K
            qkT_sb = [qkp.tile([128, t], F16, tag=f"qk{j}", name=f"qkT{j}")
                      for j in range(4)]
            # v65_sb[tb][:, h, 0:64] = V head h rows tb; [:, h, 64] = 1.0
            v65_sb = [vp.tile([128, HPC, 65], F16, tag=f"v{tb}", name=f"v65_{tb}")
                      for tb in range(n_kb)]
            # normalized attention output^T: ot_sb[j] heads (2j, 2j+1)
            ot_sb = [otp.tile([128, t], F16, tag=f"ot{j}", name=f"ot{j}")
                     for j in range(2)]
            # per-qcc softmax denominator staging
            dstage = [dsp.tile([4, QW], F16, tag="dst", name=f"dst{qcc}")
                      for qcc in range(n_qcc)]

            xT_sb = []
            for cb in range(n_cb):
                xt = xp.tile([128, t], F16, tag=f"x{cb}", name=f"xt{cb}")
                nc.sync.dma_start(xt[:], xT[cb * 128:(cb + 1) * 128, :])
                xT_sb.append(xt)

            # ---- Stage A chunks: qk^T = W_qk @ x^T (+bias) ----
            a_done = [0]

            def emit_a(ob, tb2, pool):
                pa = pool.tile([128, 512], F32, tag="pab", name="pa")
                for cb in range(n_cb):
                    nc.tensor.matmul(
                        pa[:], wqk_sb[cb][:, ob * 128:(ob + 1) * 128],
                        xT_sb[cb][:, tb2 * 512:(tb2 + 1) * 512],
                        start=(cb == 0), stop=(cb == n_cb - 1))
                nc.vector.tensor_scalar_add(
                    qkT_sb[ob][:, tb2 * 512:(tb2 + 1) * 512],
                    pa[:], bqk_sb[:, ob:ob + 1])
                a_done[0] += 1

            # ---- Stage B chunks: V = x @ W_v^T ----
            b_done = [0]

            def emit_b(tb, pool):
                pt = pool.tile([128, 512], F32, tag="pab", name="pb")
                pb = pt[:, 0:CH]
                for cb in range(n_cb):
                    nc.tensor.matmul(
                        pb, xT_sb[cb][:, tb * 128:(tb + 1) * 128],
                        wv_sb[cb][:], start=(cb == 0), stop=(cb == n_cb - 1))
                nc.vector.memset(v65_sb[tb][:, :, 64:65], 1.0)
                nc.vector.tensor_copy(
                    v65_sb[tb][:, :, 0:64],
                    pb.rearrange("p (h d) -> p h d", d=D))
                b_done[0] += 1

            # serial prefix: q heads 0-1 and k heads 0-1 for the first 1024
            # tokens (everything the first scores tiles touch); the rest of
            # stage A and all of stage B interleave into the pipeline.
            n_pre = min(2, n_t2)   # 512-chunks covering the first q-chunk
            if phase >= 1:
                with tc.tile_pool(name="psPre", bufs=2, space="PSUM") as psPre:
                    emit_a(0, 0, psPre)
                    emit_a(2, 0, psPre)
                    if n_pre > 1:
                        emit_a(0, 1, psPre)
            fillers = deque()
            if phase >= 1:
                # pop order tuned so each chunk lands before its first use:
                # B(tb) needed at step tb+LAG of (qcc0,h0); k-chunks (ob2)
                # tb2=2,3 by steps 8/12; everything else has lots of slack.
                early = []
                if n_pre > 1:
                    early.append(("a", 2, 1))
                early += [("b", 0), ("b", 1)]
                if n_t2 > 2:
                    early.append(("a", 2, 2))
                early += [("b", 2), ("b", 3)] if n_kb > 3 else []
                if n_t2 > 3:
                    early.append(("a", 2, 3))
                late = [("b", tb) for tb in range(4, n_kb)]
                late += [("a", 1, tb2) for tb2 in range(n_t2)]
                late += [("a", 3, tb2) for tb2 in range(n_t2)]
                late += [("a", 0, tb2) for tb2 in range(n_pre, n_t2)]
                late += [("a", 2, tb2) for tb2 in range(4, n_t2)]
                if phase < 2:
                    early = [it for it in early if it[0] != "b"]
                    late = [it for it in late if it[0] != "b"]
                for item in early + late:
                    if item[0] == "b" and item[1] >= n_kb:
                        continue
                    fillers.append(item)
            if (phase < 3 or SERIAL_AB) and fillers:
                with tc.tile_pool(name="psPre2", bufs=2, space="PSUM") as psPre2:
                    while fillers:
                        it = fillers.popleft()
                        if it[0] == "b":
                            emit_b(it[1], psPre2)
                        else:
                            emit_a(it[1], it[2], psPre2)

            n_qcc_eff = n_qcc if phase >= 3 else 0
            psY_ctx = ExitStack()
            psY = [None]

            # ---- Stage C: attention + output projection ----
            with tc.tile_pool(name="cTp", bufs=18) as cp, \
                 tc.tile_pool(name="ep", bufs=6) as ep, \
                 tc.tile_pool(name="pp", bufs=LAG + 2) as pp, \
                 tc.tile_pool(name="orp", bufs=5) as orp, \
                 tc.tile_pool(name="rrp", bufs=2) as rrp, \
                 tc.tile_pool(name="rbp", bufs=2) as rbp, \
                 tc.tile_pool(name="tmpp", bufs=2) as tmpp, \
                 tc.tile_pool(name="ysp", bufs=2) as ysp, \
                 tc.tile_pool(name="psS", bufs=2, space="PSUM") as psS, \
                 tc.tile_pool(name="psO", bufs=1, space="PSUM") as psO:

                psAB_ctx = ExitStack()
                psAB = [None]
                if fillers and n_qcc_eff:
                    psAB[0] = psAB_ctx.enter_context(
                        tc.tile_pool(name="psAB", bufs=2, space="PSUM"))

                def emit_outproj(qcc, ob, ns, use_scalar=False):
                    if psY[0] is None:
                        # psAB banks are free by now (all A/B chunks precede
                        # any out-projection in the filler queue)
                        psAB_ctx.close()
                        psY[0] = psY_ctx.enter_context(
                            tc.tile_pool(name="psY", bufs=2, space="PSUM"))
                    q0 = qcc * QW + ns * 512
                    Y = psY[0].tile([128, 512], F32, tag="Y", name="Y")
                    for cb in range(2):
                        nc.tensor.matmul(
                            Y[:], wo_sb[cb][:, ob * 128:(ob + 1) * 128],
                            ot_sb[cb][:, q0:q0 + 512],
                            start=(cb == 0), stop=(cb == 1))
                    ys = ysp.tile([128, 512], F16, tag="ys", name="ys")
                    if use_scalar:
                        nc.scalar.copy(ys[:], Y[:])
                    else:
                        nc.vector.tensor_copy(ys[:], Y[:])
                    nc.sync.dma_start(yT[ob * 128:(ob + 1) * 128, q0:q0 + 512],
                                      ys[:])

                def pop_filler():
                    # returns True if more fillers may be popped this step
                    it = fillers.popleft()
                    if it[0] == "b":
                        emit_b(it[1], psAB[0])
                    elif it[0] == "a":
                        emit_a(it[1], it[2], psAB[0])
                    else:
                        emit_outproj(it[1], it[2], it[3])
                        return False
                    return True

                for qcc in range(n_qcc_eff):
                    q0 = qcc * QW
                    cT_tiles = []
                    for kb in range(n_kb):
                        ct = cp.tile([128, QW], F16, tag="ct", name=f"ct{kb}")
                        nc.sync.dma_start(ct[:],
                                          cT[kb * 128:(kb + 1) * 128, q0:q0 + QW])
                        cT_tiles.append(ct)

                    oraw_heads = []
                    for h in range(HPC):
                        j, bp = h // 2, (h % 2) * 64
                        O = psO.tile([65, QW], F32, tag="O", name="O")
                        p_tiles = {}

                        def emit_attnv(kb):
                            P = p_tiles.pop(kb)
                            for ns in range(QW // 512):
                                nc.tensor.matmul(
                                    O[:, ns * 512:(ns + 1) * 512],
                                    v65_sb[kb][:, h, :],
                                    P[:, ns * 512:(ns + 1) * 512],
                                    start=(kb == 0), stop=(kb == n_kb - 1))

                        for i in range(n_kb):
                            if i >= LAG and phase >= 4:
                                emit_attnv(i - LAG)
                            for _ in range(FILL):
                                if not fillers or not pop_filler():
                                    break
                            S = psS.tile([128, QW], F32, tag="S", name="S")
                            for ns in range(QW // 512):
                                nc.tensor.matmul(
                                    S[:, ns * 512:(ns + 1) * 512],
                                    qkT_sb[2 + j][bp:bp + 64, i * 128:(i + 1) * 128],
                                    qkT_sb[j][bp:bp + 64,
                                              q0 + ns * 512:q0 + (ns + 1) * 512],
                                    start=True, stop=True)
                            E = ep.tile([128, QW], F16, tag="E", name="E")
                            nc.scalar.activation(E[:], S[:], Exp, scale=1.0)
                            P = pp.tile([128, QW], F16, tag="P", name="P")
                            p_tiles[i] = P
                            use_pool = (PMODE == 1 or
                                        (PMODE == 2 and i % 2 == 1) or
                                        (PMODE == 3 and i % 4 == 3))
                            eng = nc.gpsimd if use_pool else nc.vector
                            eng.tensor_mul(P[:], E[:], cT_tiles[i][:])
                        if phase >= 4:
                            for kb in range(n_kb - LAG, n_kb):
                                emit_attnv(kb)
                        if phase < 4:
                            continue
                        oraw = orp.tile([65, QW], F16, tag="oraw", name="oraw")
                        nc.vector.tensor_copy(oraw[:], O[:])
                        nc.sync.dma_start(dstage[qcc][h:h + 1, :],
                                          oraw[64:65, :])
                        oraw_heads.append(oraw)

                    if phase < 4:
                        continue
                    # normalization: one batched reciprocal per query chunk;
                    # engines need partition-base 0, so restage rows via DMA
                    rr = rrp.tile([4, QW], F16, tag="rr", name="rr")
                    with nc.allow_low_precision("softmax denom ~1e3, fp16 ulp ok"):
                        nc.vector.reciprocal(rr[:], dstage[qcc][:])
                    rhs_t = []
                    for h in range(HPC):
                        rh = rbp.tile([1, QW], F16, tag="rh", name="rh")
                        nc.sync.dma_start(rh[:], rr[h:h + 1, :])
                        rhs_t.append(rh)
                    for h in range(HPC):
                        rb = rbp.tile([64, QW], F16, tag="rb", name="rb")
                        nc.gpsimd.partition_broadcast(rb[:], rhs_t[h][:])
                        j, bp = h // 2, (h % 2) * 64
                        if bp == 0:
                            nc.vector.tensor_mul(ot_sb[j][0:64, q0:q0 + QW],
                                                 oraw_heads[h][0:64, :], rb[:])
                        else:
                            tmp = tmpp.tile([64, QW], F16, tag="tmp", name="tmp")
                            nc.vector.tensor_mul(tmp[:], oraw_heads[h][0:64, :],
                                                 rb[:])
                            nc.sync.dma_start(ot_sb[j][64:128, q0:q0 + QW],
                                              tmp[:])

                    if phase < 5:
                        continue
                    if qcc < n_qcc - 1:
                        for ob in range(8):
                            for ns in range(QW // 512):
                                fillers.append(("y", qcc, ob, ns))
                    else:
                        while fillers:
                            pop_filler()
                        for ob in range(8):
                            for ns in range(QW // 512):
                                emit_outproj(qcc, ob, ns, use_scalar=True)
                psY_ctx.close()

    nc.compile()
    if compile_hw:
        nc.m = get_hw_module(nc.m)
    return nc


def make_in_maps(x, cutoffs, W_qkv, b_qkv, W_out):
    """Host-side sharding: slice + transpose + cast per core.

    wqkT column blocks: ob0 = Q heads {0,1}, ob1 = Q heads {2,3},
    ob2/ob3 = same for K; all scaled by sqrt(1/8)."""
    per_batch = []
    for b in range(B):
        xT_b = np.ascontiguousarray(x[b].T).astype(np.float16)
        cT_b = np.ascontiguousarray(cutoffs[b].T).astype(np.float16)
        per_batch.append((xT_b, cT_b))

    in_maps = []
    for core in range(NCORES):
        b, hg = core // HPC, core % HPC
        ch = slice(hg * CH, (hg + 1) * CH)
        chk = slice(DIM + hg * CH, DIM + (hg + 1) * CH)
        cols = [W_qkv[ch][:128], W_qkv[ch][128:],
                W_qkv[chk][:128], W_qkv[chk][128:]]
        wqkT_c = np.ascontiguousarray(
            (np.concatenate(cols, axis=0) * RSCALE).T).astype(np.float16)
        bv = np.concatenate([b_qkv[ch], b_qkv[chk]]) * RSCALE
        bqk_c = np.ascontiguousarray(
            np.stack([bv[0:128], bv[128:256], bv[256:384], bv[384:512]],
                     axis=1)).astype(np.float32)
        wvT_c = np.ascontiguousarray(
            W_qkv[2 * DIM + hg * CH:2 * DIM + (hg + 1) * CH, :].T).astype(np.float16)
        woT_c = np.ascontiguousarray(W_out[:, ch].T).astype(np.float16)
        in_maps.append({
            "xT": per_batch[b][0], "cT": per_batch[b][1],
            "wqkT": wqkT_c, "wvT": wvT_c, "woT": woT_c, "bqk": bqk_c,
        })
    return in_maps


def kernel(x, cutoffs, W_qkv, b_qkv, W_out, b_out):
    x = np.asarray(x, dtype=np.float32)
    cutoffs = np.asarray(cutoffs, dtype=np.float32)
    W_qkv = np.asarray(W_qkv, dtype=np.float32)
    b_qkv = np.asarray(b_qkv, dtype=np.float32)
    W_out = np.asarray(W_out, dtype=np.float32)
    b_out = np.asarray(b_out, dtype=np.float32)

    if "nc" not in _cache:
        _cache["nc"] = build_kernel()
    nc = _cache["nc"]

    in_maps = make_in_maps(x, cutoffs, W_qkv, b_qkv, W_out)
    res = None
    last_err = None
    for attempt in range(3):
        try:
            res = run_bass_kernel_spmd(nc, in_maps, core_ids=list(range(NCORES)),
                                       trace=False)
            break
        except Exception as e:  # transient NRT/axon failures: retry
            last_err = e
            import time
            time.sleep(5)
    if res is None:
        raise last_err

    y = np.zeros((B, T, DIM), dtype=np.float32)
    for core in range(NCORES):
        b = core // HPC
        y[b] += res.results[core]["yT"].astype(np.float32).T
    bias_vec = W_out @ b_qkv[2 * DIM:] + b_out
    y += bias_vec[None, None, :]
    return y
